# revision 1
# baseline (speedup 1.0000x reference)
"""Trainium2 Bass kernel for nn_ComputeEnergyForce (force-field energy+force).

Strategy
--------
Data-parallel over the 16 shots across 8 NeuronCores (2 shots/core).

The hard part is the scatter-add of ~844K force contributions per shot into a
(2000, 3) per-atom force table.  Device-side scatter/gather is descriptor-bound
on TRN2, so instead the HOST pre-sorts every scatter entry by destination atom
(a pure integer permutation of the *input* index lists, shot-independent) into
an atom-major padded layout:

  - atoms are ranked by contribution count (descending) and grouped into 16
    tiles of 128 ranks; each tile is padded to its own max slot count L_i.
  - per entry we stream: dx (3 f32), one shot-dependent scalar input, and the
    shot-independent coefficients needed to build the per-entry force scalar.

On device each tile is (128 atoms x L slots); the per-entry force scalar s is
computed element-wise (DVE/ACT), and Force[atom, c] = sum_k dx[k,c]*s[k] is a
single fused `tensor_tensor_reduce` per component (reduces the whole free axis
per partition).  No PE, no PSUM, no device-side scatter.

Two entry families:
  V: vdw+coulomb pairs (2 entries/pair):  s = 12*eps*u*(1-u)/r - cc/r^2,
     u = sig6/r^6; streams per entry: dx(3), r | sig6, 12*eps, cc.
  S: bond/angle/imptors/torsion(x4 harmonics):  s = a*x + b;
     streams per entry: dx(3), x | a, b.

Energies are computed separately in natural term order (contiguous streaming).
"""

import numpy as np

import concourse.bass as bass
import concourse.bacc as bacc
import concourse.mybir as mybir
from concourse import tile
from concourse.bass_utils import run_bass_kernel_spmd

F32 = mybir.dt.float32
AF = mybir.ActivationFunctionType
ALU = mybir.AluOpType
AX = mybir.AxisListType

NS, N_ATOMS = 16, 2000
NB, NA, NV, NT, NI = 2000, 4000, 400000, 6000, 1000
CHARGE = 18.222615
NCORES = 8
SH = NS // NCORES          # shots per core
NTILES = 16                # atom tiles of 128 ranks
RANKS = NTILES * 128       # 2048 (includes 48 pad ranks)


# ----------------------------------------------------------------------------
# Host-side index preprocessing
# ----------------------------------------------------------------------------

def _sorted_tables(atom_ids):
    """Count-sorted atom-major padded placement for scatter entries.

    Returns (order, L, base, pos):
      order: (2000,) atom id per rank (rank 0 = most contributions)
      L:     per-tile slot count (multiple of 4)
      base:  per-tile entry offset into the packed table
      pos:   per-entry flat position in the packed table
    """
    counts = np.bincount(atom_ids, minlength=N_ATOMS)
    order = np.argsort(-counts, kind="stable")
    rank_of_atom = np.empty(N_ATOMS, np.int64)
    rank_of_atom[order] = np.arange(N_ATOMS)
    r = rank_of_atom[atom_ids]
    perm = np.argsort(r, kind="stable")
    rs = r[perm]
    csort = counts[order]
    starts = np.zeros(N_ATOMS + 1, np.int64)
    starts[1:] = np.cumsum(csort)
    slot_sorted = np.arange(len(rs)) - starts[rs]
    slot = np.empty_like(slot_sorted)
    slot[perm] = slot_sorted

    L = []
    for ti in range(NTILES):
        lo, hi = ti * 128, min((ti + 1) * 128, N_ATOMS)
        m = int(csort[lo:hi].max()) if lo < N_ATOMS else 0
        L.append(max(4, -(-m // 4) * 4))
    base = np.zeros(NTILES + 1, np.int64)
    base[1:] = np.cumsum([128 * l for l in L])

    ti = r >> 7
    row = r & 127
    Larr = np.asarray(L)[ti]
    pos = base[ti] + row * Larr + slot
    assert (slot < Larr).all()
    return order, L, base, pos


def _host_prep(inp):
    """Build all device-input arrays (shared across cores except shot shards)."""
    f = lambda k: np.asarray(inp[k], dtype=np.float32)
    ii = lambda k: np.asarray(inp[k], dtype=np.int64)

    length_bond = f("length_bond"); theta_angle = f("theta_angle")
    length_vdw = f("length_vdw"); sin_cos = f("sin_cos_torsion")
    cos2 = f("cos2_imptors")
    vdw14 = f("vdw14"); charge14 = f("charge14")
    pb = f("paras_bond"); pa = f("paras_angle"); pv = f("paras_vdw")
    pc = f("paras_charge"); ptor = f("paras_torsion"); pimp = f("paras_imptors")
    dlb = f("dlength_bond"); dta = f("dtheta_angle"); dlv = f("dlength_vdw")
    dtt = f("dtheta_torsion"); dci = f("dcos2_imptors")
    nb = ii("nonbonded"); b_idx = ii("bond_index"); a_idx = ii("angle_index")
    nb_idx = ii("nonbonded_index"); t_idx = ii("torsion_index")
    i_idx = ii("imptors_index")

    # --- pair parameter combinations (term order, f64 for accuracy) ---
    i, j = nb[0], nb[1]
    sigma = pv[i, 0].astype(np.float64) + pv[j, 0].astype(np.float64)
    sig6 = (sigma ** 6)
    eps = (pv[i, 1].astype(np.float64) / 10.0) * (pv[j, 1].astype(np.float64) / 10.0) * vdw14
    cc = (CHARGE / 10.0) ** 2 * pc[i].astype(np.float64) * pc[j].astype(np.float64) * charge14
    tcon = np.stack([sig6, eps, cc], axis=1).astype(np.float32)      # (NV, 3)

    # --- V family: vdw entries, 2 per pair -------------------------------
    av = nb_idx.reshape(-1)                       # (2*NV,) atom per entry
    tv = np.arange(2 * NV) >> 1                   # term per entry
    orderV, LV, baseV, posV = _sorted_tables(av)
    TOTV = int(baseV[-1])
    # dx in fp16 plane-major (shot, component, pos) so each STT input is a
    # contiguous step-1 fp16 run; r stays f32 (feeds reciprocal_approx_fast)
    vdx = np.zeros((NS, 3, TOTV), np.float16)
    vdx[:, :, posV] = dlv.reshape(NS, 2 * NV, 3).transpose(0, 2, 1)
    vr = np.ones((NS, TOTV), np.float32)          # pad r = 1 (avoid 1/0)
    vr[:, posV] = length_vdw[:, tv]
    vcon = np.zeros((3, TOTV), np.float32)
    vcon[0, posV] = sig6[tv]
    vcon[1, posV] = 12.0 * eps[tv]
    vcon[2, posV] = cc[tv]

    # --- S family: bond / angle / imptors / torsion-expanded -------------
    K = pb[:, 0].astype(np.float64) * 100.0
    r0 = pb[:, 1].astype(np.float64)
    Ka = pa[:, 0].astype(np.float64) * 10.0
    th0 = pa[:, 1].astype(np.float64) * (np.pi / 10.0)
    ki = pimp[:, 0].astype(np.float64)
    coeff = ptor.astype(np.float64) * np.arange(1, 5, dtype=np.float64)[None]

    e_b = np.arange(2 * NB) >> 1
    e_a = np.arange(3 * NA) // 3
    e_i = np.arange(4 * NI) >> 2
    ntt = 4 * NT                                   # torsion term-slot entries
    tt = np.arange(ntt) >> 2                       # torsion term per entry
    tt_rep = np.repeat(tt, 4)                      # expanded x4 harmonics
    et_rep = np.repeat(np.arange(ntt), 4)
    n_rep = np.tile(np.arange(4), ntt)

    aS = np.concatenate([
        b_idx.reshape(-1), a_idx.reshape(-1), i_idx.reshape(-1),
        np.repeat(t_idx.reshape(-1), 4),
    ])
    caS = np.concatenate([
        (2.0 * K)[e_b], (2.0 * Ka)[e_a], np.zeros(4 * NI),
        -coeff[tt_rep, n_rep],
    ]).astype(np.float32)
    cbS = np.concatenate([
        (-2.0 * K * r0)[e_b], (-2.0 * Ka * th0)[e_a], -ki[e_i],
        np.zeros(4 * ntt),
    ]).astype(np.float32)

    # x gather (shot-dependent): indices into concatenated per-shot sources
    off_th = NB
    off_sc = NB + NA
    off_z = NB + NA + NT * 8
    xiS = np.concatenate([
        e_b, off_th + e_a, np.full(4 * NI, off_z, np.int64),
        off_sc + tt_rep * 8 + 2 * n_rep,
    ])
    XS = np.concatenate([
        length_bond, theta_angle, sin_cos.reshape(NS, -1),
        np.zeros((NS, 1), np.float32),
    ], axis=1)
    sxS = XS[:, xiS]                               # (NS, NES)

    dxS = np.concatenate([
        dlb.reshape(NS, 2 * NB, 3), dta.reshape(NS, 3 * NA, 3),
        dci.reshape(NS, 4 * NI, 3),
        np.repeat(dtt.reshape(NS, ntt, 3), 4, axis=1),
    ], axis=1)

    orderS, LS, baseS, posS = _sorted_tables(aS)
    TOTS = int(baseS[-1])
    sdx = np.zeros((NS, 3, TOTS), np.float16)
    sdx[:, :, posS] = dxS.transpose(0, 2, 1)
    sx = np.zeros((NS, TOTS), np.float32)
    sx[:, posS] = sxS
    scon = np.zeros((2, TOTS), np.float32)
    scon[0, posS] = caS
    scon[1, posS] = cbS

    # --- small-term parameter packs --------------------------------------
    bc = np.stack([K, r0], axis=1).astype(np.float32)          # (NB, 2)
    ac = np.stack([Ka, th0], axis=1).astype(np.float32)        # (NA, 2)

    host = dict(
        lb=length_bond, th=theta_angle, rv=length_vdw,
        sc=sin_cos.reshape(NS, -1), c2=cos2,
        bc=bc, ac=ac, pt=ptor, ki=pimp[:, 0].astype(np.float32),
        tcon=tcon, vdx=vdx, vr=vr, vcon=vcon, sdx=sdx, sx=sx, scon=scon,
    )
    meta = dict(LV=LV, LS=LS, baseV=baseV, baseS=baseS,
                TOTV=TOTV, TOTS=TOTS, orderV=orderV, orderS=orderS)
    return host, meta


# ----------------------------------------------------------------------------
# Device kernel
# ----------------------------------------------------------------------------

_NC_CACHE = {}


def _build_nc(LV, LS, baseV, baseS, TOTV, TOTS, blocks=("sm", "ev", "vf", "sf")):
    key = (tuple(LV), tuple(LS), tuple(blocks))
    if key in _NC_CACHE:
        return _NC_CACHE[key]

    nc = bacc.Bacc("TRN2")
    F16 = mybir.dt.float16
    dp = lambda n, s, o=False: nc.declare_dram_parameter(n, list(s), F32, isOutput=o)
    dph = lambda n, s: nc.declare_dram_parameter(n, list(s), F16, isOutput=False)

    t_lb = dp("lb", (SH, NB)); t_th = dp("th", (SH, NA))
    t_rv = dp("rv", (SH, NV)); t_sc = dp("sc", (SH, NT * 8))
    t_c2 = dp("c2", (SH, NI))
    t_bc = dp("bc", (NB, 2)); t_ac = dp("ac", (NA, 2))
    t_pt = dp("pt", (NT, 4)); t_ki = dp("ki", (NI,))
    t_tc = dp("tcon", (NV, 3))
    t_vdx = dph("vdx", (SH, 3, TOTV)); t_vr = dp("vr", (SH, TOTV))
    t_vc = dp("vcon", (3, TOTV))
    t_sdx = dph("sdx", (SH, 3, TOTS)); t_sx = dp("sx", (SH, TOTS))
    t_scn = dp("scon", (2, TOTS))

    o_eb = dp("e_bond", (SH, NB), True); o_ea = dp("e_angle", (SH, NA), True)
    o_ev = dp("e_vdw", (SH, NV), True); o_ec = dp("e_charge", (SH, NV), True)
    o_et = dp("e_tors", (SH, NT), True); o_ei = dp("e_impt", (SH, NI), True)
    o_fv = dp("f_v", (SH, RANKS, 3), True)
    o_fs = dp("f_s", (SH, RANKS, 3), True)

    A = bass.AP  # AP(tensor, offset, [[step, count], ...])

    with tile.TileContext(nc) as tc:
        with tc.tile_pool(name="io", bufs=2) as io, \
             tc.tile_pool(name="scr", bufs=2) as scr, \
             tc.tile_pool(name="acc", bufs=4) as acc:

            def ttr(dead, dx_ap, s_ap, accum):
                # fused multiply + free-axis sum (tensor_tensor_reduce is
                # broken on HW via this runtime; InstTensorScalarPtr works)
                nc.vector.scalar_tensor_tensor(
                    out=dead[:], in0=dx_ap, scalar=1.0, in1=s_ap,
                    op0=ALU.mult, op1=ALU.mult, accum_out=accum)

            if "sm" in blocks:
            # ---------------- small-term energies ----------------
                # bond
                bct = io.tile([125, 16, 2], F32, tag="bct")
                nc.scalar.dma_start(bct[:], A(t_bc, 0, [[32, 125], [2, 16], [1, 2]]))
                for sh in range(SH):
                    lbt = io.tile([125, 16], F32, tag="lbt")
                    nc.sync.dma_start(lbt[:], A(t_lb, sh * NB, [[16, 125], [1, 16]]))
                    d = scr.tile([125, 16], F32, tag="sm0")
                    nc.vector.tensor_sub(d[:], lbt[:], bct[:, :, 1])
                    kd = scr.tile([125, 16], F32, tag="sm1")
                    nc.vector.tensor_mul(kd[:], d[:], bct[:, :, 0])
                    e = scr.tile([125, 16], F32, tag="sm2")
                    nc.vector.tensor_mul(e[:], kd[:], d[:])
                    nc.gpsimd.dma_start(A(o_eb, sh * NB, [[16, 125], [1, 16]]), e[:])
                # angle
                act = io.tile([125, 32, 2], F32, tag="act")
                nc.scalar.dma_start(act[:], A(t_ac, 0, [[64, 125], [2, 32], [1, 2]]))
                for sh in range(SH):
                    tht = io.tile([125, 32], F32, tag="tht")
                    nc.sync.dma_start(tht[:], A(t_th, sh * NA, [[32, 125], [1, 32]]))
                    d = scr.tile([125, 32], F32, tag="sm0")
                    nc.vector.tensor_sub(d[:], tht[:], act[:, :, 1])
                    kd = scr.tile([125, 32], F32, tag="sm1")
                    nc.vector.tensor_mul(kd[:], d[:], act[:, :, 0])
                    e = scr.tile([125, 32], F32, tag="sm2")
                    nc.vector.tensor_mul(e[:], kd[:], d[:])
                    nc.gpsimd.dma_start(A(o_ea, sh * NA, [[32, 125], [1, 32]]), e[:])
                # torsion energy
                ptt = io.tile([125, 48, 4], F32, tag="ptt")
                nc.scalar.dma_start(ptt[:], A(t_pt, 0, [[192, 125], [4, 48], [1, 4]]))
                for sh in range(SH):
                    sct = io.tile([125, 48, 8], F32, tag="sct")
                    nc.sync.dma_start(
                        sct[:], A(t_sc, sh * NT * 8, [[384, 125], [8, 48], [1, 8]]))
                    cos_ap = A(sct[:].tensor, sct[:].offset + 1,
                               [sct[:].ap[0], [8, 48], [2, 4]])
                    prod = scr.tile([125, 48, 4], F32, tag="sm0")
                    nc.vector.tensor_mul(prod[:], cos_ap, ptt[:])
                    e = scr.tile([125, 48], F32, tag="sm2")
                    nc.vector.reduce_sum(e[:], prod[:], axis=AX.X)
                    nc.gpsimd.dma_start(A(o_et, sh * NT, [[48, 125], [1, 48]]), e[:])
                # improper torsion energy
                kit = io.tile([125, 8], F32, tag="kit")
                nc.scalar.dma_start(kit[:], A(t_ki, 0, [[8, 125], [1, 8]]))
                for sh in range(SH):
                    c2t = io.tile([125, 8], F32, tag="c2t")
                    nc.sync.dma_start(c2t[:], A(t_c2, sh * NI, [[8, 125], [1, 8]]))
                    t1 = scr.tile([125, 8], F32, tag="sm0")
                    nc.scalar.activation(t1[:], c2t[:], AF.Copy, bias=1.0, scale=-1.0)
                    e = scr.tile([125, 8], F32, tag="sm2")
                    nc.vector.tensor_mul(e[:], t1[:], kit[:])
                    nc.gpsimd.dma_start(A(o_ei, sh * NI, [[8, 125], [1, 8]]), e[:])

            if "ev" in blocks:
            # ---------------- vdw/coulomb energies (term order) ----------
            # Both shots merged into one (128, SH, CH) op stream; per-pair
            # constants broadcast across the shot axis with step-0 APs.
                NCH, CH = 5, 625           # 400000 = 128 * 3125 = 128 * 5 * 625
                for k in range(NCH):
                    tct = io.tile([128, CH, 3], F32, tag="tct")
                    nc.scalar.dma_start(
                        tct[:], A(t_tc, 625 * k * 3, [[3125 * 3, 128], [3, CH], [1, 3]]))
                    tb = lambda c: A(tct[:].tensor, tct[:].offset + c,
                                     [tct[:].ap[0], [0, SH], [3, CH]])
                    rvt = io.tile([128, SH, CH], F32, tag="rvt")
                    for sh in range(SH):
                        nc.sync.dma_start(
                            rvt[:, sh], A(t_rv, sh * NV + 625 * k, [[3125, 128], [1, CH]]))
                    w = scr.tile([128, SH, CH], F32, tag="w")
                    nc.vector.reciprocal_approx_fast(out=w[:], in_=rvt[:])
                    w2 = scr.tile([128, SH, CH], F32, tag="w2")
                    nc.scalar.square(w2[:], w[:])
                    w4 = scr.tile([128, SH, CH], F32, tag="w4")
                    nc.scalar.square(w4[:], w2[:])
                    w6 = scr.tile([128, SH, CH], F32, tag="w6")
                    nc.vector.tensor_mul(w6[:], w2[:], w4[:])
                    u = scr.tile([128, SH, CH], F32, tag="u")
                    nc.vector.tensor_mul(u[:], w6[:], tb(0))
                    m = scr.tile([128, SH, CH], F32, tag="m1")
                    nc.vector.tensor_mul(m[:], u[:], tb(1))
                    t2 = scr.tile([128, SH, CH], F32, tag="a1")
                    nc.scalar.activation(t2[:], u[:], AF.Copy, bias=-2.0, scale=1.0)
                    ev = scr.tile([128, SH, CH], F32, tag="p")
                    nc.gpsimd.tensor_mul(ev[:], m[:], t2[:])
                    ecg = scr.tile([128, SH, CH], F32, tag="a4")
                    nc.vector.tensor_mul(ecg[:], w[:], tb(2))
                    for sh in range(SH):
                        nc.gpsimd.dma_start(
                            A(o_ev, sh * NV + 625 * k, [[3125, 128], [1, CH]]), ev[:, sh])
                        nc.gpsimd.dma_start(
                            A(o_ec, sh * NV + 625 * k, [[3125, 128], [1, CH]]), ecg[:, sh])

            # ---------------- force: V family ----------------------------
            for ti in range(NTILES):
                if "vf" not in blocks and "sf" not in blocks:
                    break
                if "vf" in blocks:
                    L = LV[ti]
                    bV = int(baseV[ti])
                    vdxt = io.tile([128, SH, 3, L], F16, tag="vdx")
                    vrt = io.tile([128, SH, L], F32, tag="vr")
                    for sh in range(SH):
                        nc.sync.dma_start(
                            vdxt[:, sh], A(t_vdx, sh * 3 * TOTV + bV,
                                           [[L, 128], [TOTV, 3], [1, L]]))
                        nc.sync.dma_start(
                            vrt[:, sh], A(t_vr, sh * TOTV + bV, [[L, 128], [1, L]]))
                    vct = io.tile([128, 3, L], F32, tag="vcon")
                    nc.scalar.dma_start(
                        vct[:], A(t_vc, bV, [[L, 128], [TOTV, 3], [1, L]]))
                    vb = lambda c: A(vct[:, c].tensor, vct[:, c].offset,
                                     [vct[:, c].ap[0], [0, SH], [1, L]])
                    facc = acc.tile([128, SH * 3], F32, tag="facc")
                    w = scr.tile([128, SH, L], F32, tag="w")
                    nc.vector.reciprocal_approx_fast(out=w[:], in_=vrt[:])
                    w2 = scr.tile([128, SH, L], F32, tag="w2")
                    nc.scalar.square(w2[:], w[:])
                    w4 = scr.tile([128, SH, L], F32, tag="w4")
                    nc.scalar.square(w4[:], w2[:])
                    w6 = scr.tile([128, SH, L], F32, tag="w6")
                    nc.vector.tensor_mul(w6[:], w2[:], w4[:])
                    u = scr.tile([128, SH, L], F32, tag="u")
                    nc.vector.tensor_mul(u[:], w6[:], vb(0))
                    m1 = scr.tile([128, SH, L], F32, tag="m1")
                    nc.scalar.activation(m1[:], u[:], AF.Copy, bias=1.0, scale=-1.0)
                    a1 = scr.tile([128, SH, L], F32, tag="a1")
                    nc.vector.tensor_mul(a1[:], u[:], w[:])
                    P = scr.tile([128, SH, L], F32, tag="p")
                    nc.vector.tensor_mul(P[:], a1[:], vb(1))
                    a4 = scr.tile([128, SH, L], F32, tag="a4")
                    nc.vector.tensor_mul(a4[:], w2[:], vb(2))
                    pm = scr.tile([128, SH, L], F32, tag="pm")
                    nc.gpsimd.tensor_mul(pm[:], P[:], m1[:])
                    s = scr.tile([128, SH, L], F32, tag="s")
                    nc.gpsimd.tensor_sub(s[:], pm[:], a4[:])
                    for sh in range(SH):
                        for c in range(3):
                            dead = scr.tile([128, L], F32, tag="dead")
                            ttr(dead, vdxt[:, sh, c], s[:, sh],
                                facc[:, sh * 3 + c:sh * 3 + c + 1])
                    nc.gpsimd.dma_start(
                        A(o_fv, ti * 128 * 3, [[3, 128], [RANKS * 3, SH], [1, 3]]),
                        facc[:].rearrange("p (s c) -> p s c", s=SH))

                # ---------------- force: S family ------------------------
                if "sf" not in blocks:
                    continue
                Ls = LS[ti]
                bS = int(baseS[ti])
                sdxt = io.tile([128, SH, 3, Ls], F16, tag="sdx")
                sxt = io.tile([128, SH, Ls], F32, tag="sx")
                for sh in range(SH):
                    nc.sync.dma_start(
                        sdxt[:, sh], A(t_sdx, sh * 3 * TOTS + bS,
                                       [[Ls, 128], [TOTS, 3], [1, Ls]]))
                    nc.sync.dma_start(
                        sxt[:, sh], A(t_sx, sh * TOTS + bS, [[Ls, 128], [1, Ls]]))
                sct2 = io.tile([128, 2, Ls], F32, tag="scon")
                nc.scalar.dma_start(
                    sct2[:], A(t_scn, bS, [[Ls, 128], [TOTS, 2], [1, Ls]]))
                sb_ = lambda c: A(sct2[:, c].tensor, sct2[:, c].offset,
                                  [sct2[:, c].ap[0], [0, SH], [1, Ls]])
                sacc = acc.tile([128, SH * 3], F32, tag="sacc")
                t1 = scr.tile([128, SH, Ls], F32, tag="w")
                nc.vector.tensor_mul(t1[:], sxt[:], sb_(0))
                s2 = scr.tile([128, SH, Ls], F32, tag="s")
                nc.vector.tensor_add(s2[:], t1[:], sb_(1))
                for sh in range(SH):
                    for c in range(3):
                        dead = scr.tile([128, Ls], F32, tag="dead")
                        ttr(dead, sdxt[:, sh, c], s2[:, sh],
                            sacc[:, sh * 3 + c:sh * 3 + c + 1])
                nc.gpsimd.dma_start(
                    A(o_fs, ti * 128 * 3, [[3, 128], [RANKS * 3, SH], [1, 3]]),
                    sacc[:].rearrange("p (s c) -> p s c", s=SH))

    nc.finalize()
    _NC_CACHE[key] = nc
    return nc


# ----------------------------------------------------------------------------
# Entry points
# ----------------------------------------------------------------------------

def _in_maps(host, meta):
    maps = []
    for c in range(NCORES):
        sl = slice(c * SH, (c + 1) * SH)
        maps.append({
            "lb": host["lb"][sl], "th": host["th"][sl], "rv": host["rv"][sl],
            "sc": host["sc"][sl], "c2": host["c2"][sl],
            "bc": host["bc"], "ac": host["ac"], "pt": host["pt"],
            "ki": host["ki"], "tcon": host["tcon"],
            "vdx": host["vdx"][sl], "vr": host["vr"][sl],
            "vcon": host["vcon"], "sdx": host["sdx"][sl],
            "sx": host["sx"][sl], "scon": host["scon"],
        })
    return maps


def _assemble(results, meta):
    orderV, orderS = meta["orderV"], meta["orderS"]
    e_bond = np.concatenate([r["e_bond"] for r in results], axis=0)
    e_angle = np.concatenate([r["e_angle"] for r in results], axis=0)
    e_vdw = np.concatenate([r["e_vdw"] for r in results], axis=0)
    e_charge = np.concatenate([r["e_charge"] for r in results], axis=0)
    e_tors = np.concatenate([r["e_tors"] for r in results], axis=0)
    e_impt = np.concatenate([r["e_impt"] for r in results], axis=0)
    f_v = np.concatenate([r["f_v"] for r in results], axis=0)  # (NS,RANKS,3)
    f_s = np.concatenate([r["f_s"] for r in results], axis=0)
    force = np.zeros((NS, N_ATOMS, 3), np.float32)
    force[:, orderV] = f_v[:, :N_ATOMS]
    fs = np.zeros((NS, N_ATOMS, 3), np.float32)
    fs[:, orderS] = f_s[:, :N_ATOMS]
    force += fs
    return np.concatenate([
        e_bond, e_angle, np.zeros((NS, 1), np.float32), e_vdw, e_charge,
        e_tors, e_impt, force.reshape(NS, -1),
    ], axis=1)


def run(inputs, trace=False):
    host, meta = _host_prep(inputs)
    nc = _build_nc(meta["LV"], meta["LS"], meta["baseV"], meta["baseS"],
                   meta["TOTV"], meta["TOTS"])
    res = run_bass_kernel_spmd(nc, _in_maps(host, meta), list(range(NCORES)),
                               trace=trace)
    return _assemble(res.results, meta), res


def kernel(**inputs) -> np.ndarray:
    out, _ = run(inputs)
    return out



# revision 4
# speedup vs baseline: 1.3707x; 1.3707x over previous
"""Trainium2 Bass kernel for nn_ComputeEnergyForce (force-field energy+force).

Strategy (v2)
-------------
ENTRY-sharding: the ~844K scatter entries (and the pair list) are split
across the 8 cores; every core processes ALL 16 shots for its 1/8 slice.
Compared to shot-sharding this loads the shot-independent per-entry
constants once per slice instead of 8x.

Scatter-add layout (host-prepared, as v1): atoms are count-sorted into
2048 ranks; tile t covers ranks [128t, 128t+128).  Core c owns tile pair
(c, 15-c); slot widths LA/LB are common across cores (SPMD).  Per entry
we stream dx (3x fp16), r (fp16), and 3 fp16 constants; on device
Force[row, sh, c] = sum_k dx[k]*sg[k] via scalar_tensor_tensor accum.

Reciprocal powers via one ACT table set (natural_log_exp_and_others):
  lr = Ln(r); w = Exp(-lr); w6 = Exp(-6 lr)        (Scalar engine)
  u  = w6*S;  z=(u-1)u; z3=(u-2)u; q1=z*A          (Vector, fp16 2x)
  t2 = C*w (=E_charge); d = t2+q1                  (GpSimd)
  sg = w*d (= -s_vdw-coul);  e2 = z3*A (=12*E_vdw) (Vector)
Energies are emitted per-entry in the scatter layout and un-permuted on
the host (pure gather); no separate term-order vdw pass exists.

S family (bond/angle/imptors/torsion-harmonics): sg = a*x + b, same
scatter layout.  Small term-order energy passes for bond/angle/torsion/
imptors energies only.  All HBM streams fp16; force accum f32.
"""

import numpy as np

import concourse.bass as bass
import concourse.bacc as bacc
import concourse.mybir as mybir
from concourse import tile
from concourse.bass_utils import run_bass_kernel_spmd

F32 = mybir.dt.float32
F16 = mybir.dt.float16
AF = mybir.ActivationFunctionType
ALU = mybir.AluOpType
AX = mybir.AxisListType

NS, N_ATOMS = 16, 2000
NB, NA, NV, NT, NI = 2000, 4000, 400000, 6000, 1000
CHARGE = 18.222615
NCORES = 8
G = 4                      # shots per group (V family)
NSG = NS // G              # shot groups
NBc, NAc, NTc, NIc = NB // 8, NA // 8, NT // 8, NI // 8

_r4 = lambda x: max(4, -(-int(x) // 4) * 4)


# ----------------------------------------------------------------------------
# Host-side index preprocessing
# ----------------------------------------------------------------------------

def _rank_tables(atom_ids):
    """Count-sort atoms -> ranks; return per-entry (rank, slot) + order."""
    counts = np.bincount(atom_ids, minlength=N_ATOMS)
    order = np.argsort(-counts, kind="stable")
    rank_of_atom = np.empty(N_ATOMS, np.int64)
    rank_of_atom[order] = np.arange(N_ATOMS)
    r = rank_of_atom[atom_ids]
    perm = np.argsort(r, kind="stable")
    rs = r[perm]
    csort = counts[order]
    starts = np.zeros(N_ATOMS + 1, np.int64)
    starts[1:] = np.cumsum(csort)
    slot_sorted = np.arange(len(rs)) - starts[rs]
    slot = np.empty_like(slot_sorted)
    slot[perm] = slot_sorted
    return order, csort, r, slot


def _core_pos(rank, slot, LA, LB):
    """Map global rank+slot -> (core, local flat position)."""
    t = rank >> 7
    row = rank & 127
    core = np.where(t < 8, t, 15 - t)
    in_b = (t >= 8).astype(np.int64)
    pos = np.where(in_b == 0, row * LA + slot, 128 * LA + row * LB + slot)
    return core, pos, in_b, row


def _host_prep(inp):
    f = lambda k: np.asarray(inp[k], dtype=np.float32)
    ii = lambda k: np.asarray(inp[k], dtype=np.int64)
    h16 = np.float16

    length_bond = f("length_bond"); theta_angle = f("theta_angle")
    length_vdw = f("length_vdw"); sin_cos = f("sin_cos_torsion")
    cos2 = f("cos2_imptors")
    vdw14 = f("vdw14"); charge14 = f("charge14")
    pb = f("paras_bond"); pa = f("paras_angle"); pv = f("paras_vdw")
    pc = f("paras_charge"); ptor = f("paras_torsion"); pimp = f("paras_imptors")
    dlb = f("dlength_bond"); dta = f("dtheta_angle"); dlv = f("dlength_vdw")
    dtt = f("dtheta_torsion"); dci = f("dcos2_imptors")
    nb = ii("nonbonded"); b_idx = ii("bond_index"); a_idx = ii("angle_index")
    nb_idx = ii("nonbonded_index"); t_idx = ii("torsion_index")
    i_idx = ii("imptors_index")

    # --- pair constants (f64 for accuracy) ---
    i, j = nb[0], nb[1]
    sig6 = (pv[i, 0].astype(np.float64) + pv[j, 0].astype(np.float64)) ** 6
    A12 = 12.0 * (pv[i, 1].astype(np.float64) / 10.0) * (pv[j, 1] / 10.0) * vdw14
    Ccc = (CHARGE / 10.0) ** 2 * pc[i].astype(np.float64) * pc[j] * charge14

    # ---------------- V family ----------------
    EV = 2 * NV
    av = nb_idx.reshape(-1)
    tv = np.arange(EV) >> 1
    orderV, csortV, rkV, slV = _rank_tables(av)
    LA = _r4(csortV[0]); LB = _r4(csortV[1024])
    TV = 128 * (LA + LB)
    ceV, cpV, _, _ = _core_pos(rkV, slV, LA, LB)

    vdx = np.zeros((NCORES, NS, 3, TV), h16)
    vdx[ceV, :, :, cpV] = dlv.reshape(NS, EV, 3).transpose(1, 0, 2)
    vr = np.ones((NCORES, NS, TV), h16)
    vr[ceV, :, cpV] = length_vdw[:, tv].T
    vcon = np.zeros((NCORES, 3, TV), h16)
    vcon[ceV, 0, cpV] = sig6[tv]
    vcon[ceV, 1, cpV] = A12[tv]
    vcon[ceV, 2, cpV] = Ccc[tv]

    # energy un-permute (use end-0 entry of each pair)
    e0 = np.arange(0, EV, 2)
    ev_core, ev_pos = ceV[e0], cpV[e0]

    # ---------------- S family ----------------
    K = pb[:, 0].astype(np.float64) * 100.0
    r0 = pb[:, 1].astype(np.float64)
    Ka = pa[:, 0].astype(np.float64) * 10.0
    th0 = pa[:, 1].astype(np.float64) * (np.pi / 10.0)
    ki = pimp[:, 0].astype(np.float64)
    coeff = ptor.astype(np.float64) * np.arange(1, 5, dtype=np.float64)[None]

    e_b = np.arange(2 * NB) >> 1
    e_a = np.arange(3 * NA) // 3
    e_i = np.arange(4 * NI) >> 2
    ntt = 4 * NT
    tt = np.arange(ntt) >> 2
    tt_rep = np.repeat(tt, 4)
    n_rep = np.tile(np.arange(4), ntt)

    aS = np.concatenate([
        b_idx.reshape(-1), a_idx.reshape(-1), i_idx.reshape(-1),
        np.repeat(t_idx.reshape(-1), 4),
    ])
    caS = np.concatenate([
        (2.0 * K)[e_b], (2.0 * Ka)[e_a], np.zeros(4 * NI),
        -coeff[tt_rep, n_rep],
    ])
    cbS = np.concatenate([
        (-2.0 * K * r0)[e_b], (-2.0 * Ka * th0)[e_a], -ki[e_i],
        np.zeros(4 * ntt),
    ])
    off_th, off_sc, off_z = NB, NB + NA, NB + NA + NT * 8
    xiS = np.concatenate([
        e_b, off_th + e_a, np.full(4 * NI, off_z, np.int64),
        off_sc + tt_rep * 8 + 2 * n_rep,
    ])
    XS = np.concatenate([
        length_bond, theta_angle, sin_cos.reshape(NS, -1),
        np.zeros((NS, 1), np.float32),
    ], axis=1)
    dxS = np.concatenate([
        dlb.reshape(NS, 2 * NB, 3), dta.reshape(NS, 3 * NA, 3),
        dci.reshape(NS, 4 * NI, 3),
        np.repeat(dtt.reshape(NS, ntt, 3), 4, axis=1),
    ], axis=1)

    orderS, csortS, rkS, slS = _rank_tables(aS)
    LSA = _r4(csortS[0]); LSB = _r4(csortS[1024])
    TS = 128 * (LSA + LSB)
    ceS, cpS, _, _ = _core_pos(rkS, slS, LSA, LSB)

    sdx = np.zeros((NCORES, NS, 3, TS), h16)
    sdx[ceS, :, :, cpS] = dxS.transpose(1, 0, 2)
    sx = np.zeros((NCORES, NS, TS), h16)
    sx[ceS, :, cpS] = XS[:, xiS].T
    scon = np.zeros((NCORES, 2, TS), h16)
    scon[ceS, 0, cpS] = caS
    scon[ceS, 1, cpS] = cbS

    # ---------------- small term-order energy slices ----------------
    smalls = []
    for c in range(NCORES):
        sb, sa = slice(c * NBc, (c + 1) * NBc), slice(c * NAc, (c + 1) * NAc)
        st, si = slice(c * NTc, (c + 1) * NTc), slice(c * NIc, (c + 1) * NIc)
        smalls.append(dict(
            lb=length_bond[:, sb].astype(h16),
            th=theta_angle[:, sa].astype(h16),
            sc=np.ascontiguousarray(sin_cos[:, st]).astype(h16),
            c2=cos2[:, si].astype(h16),
            bK=np.tile(K[sb], NS).astype(h16),
            bR=np.tile(r0[sb], NS).astype(h16),
            aK=np.tile(Ka[sa], NS).astype(h16),
            aT=np.tile(th0[sa], NS).astype(h16),
            pt=np.tile(ptor[st].reshape(-1), NS).astype(h16),
            kT=np.tile(ki[si], NS).astype(h16),
        ))

    host = dict(vdx=vdx, vr=vr, vcon=vcon, sdx=sdx, sx=sx, scon=scon,
                smalls=smalls)
    meta = dict(LA=LA, LB=LB, TV=TV, LSA=LSA, LSB=LSB, TS=TS,
                orderV=orderV, orderS=orderS,
                ev_core=ev_core, ev_pos=ev_pos)
    return host, meta


# ----------------------------------------------------------------------------
# Device kernel
# ----------------------------------------------------------------------------

_NC_CACHE = {}


def _build_nc(LA, LB, TV, LSA, LSB, TS):
    key = (LA, LB, LSA, LSB)
    if key in _NC_CACHE:
        return _NC_CACHE[key]

    nc = bacc.Bacc("TRN2")
    dp = lambda n, s, dt=F16, o=False: nc.declare_dram_parameter(
        n, list(s), dt, isOutput=o)

    t_vdx = dp("vdx", (NS, 3, TV)); t_vr = dp("vr", (NS, TV))
    t_vc = dp("vcon", (3, TV))
    t_sdx = dp("sdx", (NS, 3, TS)); t_sx = dp("sx", (NS, TS))
    t_scn = dp("scon", (2, TS))
    t_lb = dp("lb", (NS, NBc)); t_th = dp("th", (NS, NAc))
    t_sc = dp("sc", (NS, NTc * 8)); t_c2 = dp("c2", (NS, NIc))
    t_bK = dp("bK", (NS * NBc,)); t_bR = dp("bR", (NS * NBc,))
    t_aK = dp("aK", (NS * NAc,)); t_aT = dp("aT", (NS * NAc,))
    t_pt = dp("pt", (NS * NTc * 4,)); t_kT = dp("kT", (NS * NIc,))

    o_ev = dp("o_ev", (NS, TV), o=True)
    o_ec = dp("o_ec", (NS, TV), o=True)
    o_fv = dp("o_fv", (2, 128, NS, 3), F32, True)
    o_fs = dp("o_fs", (2, 128, NS, 3), F32, True)
    o_eb = dp("o_eb", (NS, NBc), o=True); o_ea = dp("o_ea", (NS, NAc), o=True)
    o_et = dp("o_et", (NS, NTc), o=True); o_ei = dp("o_ei", (NS, NIc), o=True)

    A = bass.AP

    with tile.TileContext(nc) as tc:
        with tc.tile_pool(name="io", bufs=2) as io, \
             tc.tile_pool(name="scr", bufs=2) as scr, \
             tc.tile_pool(name="acc", bufs=2) as acc:

            # ================= V family =================
            for sl, (L, base) in enumerate(((LA, 0), (LB, 128 * LA))):
                vct = io.tile([128, 3, L], F16, tag=f"vct{sl}")
                nc.sync.dma_start(vct[:], A(t_vc, base, [[L, 128], [TV, 3], [1, L]]))
                bc = lambda c: A(vct[:, c].tensor, vct[:, c].offset,
                                 [vct[:, c].ap[0], [0, G], [1, L]])
                facc = acc.tile([128, NS, 3], F32, tag=f"faccV{sl}")
                for g in range(NSG):
                    sh0 = g * G
                    vdxt = io.tile([128, G, 3, L], F16, tag="vdxt")
                    nc.sync.dma_start(
                        vdxt[:], A(t_vdx, sh0 * 3 * TV + base,
                                   [[L, 128], [3 * TV, G], [TV, 3], [1, L]]))
                    vrt = io.tile([128, G, L], F16, tag="vrt")
                    nc.sync.dma_start(
                        vrt[:], A(t_vr, sh0 * TV + base,
                                  [[L, 128], [TV, G], [1, L]]))
                    lr = scr.tile([128, G, L], F32, tag="lr")
                    nc.scalar.activation(lr[:], vrt[:], AF.Ln)
                    w = scr.tile([128, G, L], F16, tag="w")
                    nc.scalar.activation(w[:], lr[:], AF.Exp, scale=-1.0)
                    w6 = scr.tile([128, G, L], F16, tag="w6")
                    nc.scalar.activation(w6[:], lr[:], AF.Exp, scale=-6.0)
                    u = scr.tile([128, G, L], F16, tag="u")
                    nc.vector.tensor_mul(u[:], w6[:], bc(0))
                    z = scr.tile([128, G, L], F16, tag="z")
                    nc.vector.scalar_tensor_tensor(
                        out=z[:], in0=u[:], scalar=1.0, in1=u[:],
                        op0=ALU.subtract, op1=ALU.mult)
                    z3 = scr.tile([128, G, L], F16, tag="z3")
                    nc.vector.scalar_tensor_tensor(
                        out=z3[:], in0=u[:], scalar=2.0, in1=u[:],
                        op0=ALU.subtract, op1=ALU.mult)
                    q1 = scr.tile([128, G, L], F16, tag="q1")
                    nc.vector.tensor_mul(q1[:], z[:], bc(1))
                    t2 = scr.tile([128, G, L], F16, tag="t2")
                    nc.vector.tensor_mul(t2[:], w[:], bc(2))
                    d = scr.tile([128, G, L], F16, tag="d")
                    nc.gpsimd.tensor_add(d[:], t2[:], q1[:])
                    sg = scr.tile([128, G, L], F16, tag="sg")
                    nc.gpsimd.tensor_mul(sg[:], w[:], d[:])
                    e2 = scr.tile([128, G, L], F16, tag="e2")
                    nc.vector.tensor_mul(e2[:], z3[:], bc(1))
                    nc.scalar.dma_start(
                        A(o_ec, sh0 * TV + base, [[L, 128], [TV, G], [1, L]]),
                        t2[:])
                    nc.scalar.dma_start(
                        A(o_ev, sh0 * TV + base, [[L, 128], [TV, G], [1, L]]),
                        e2[:])
                    for s in range(G):
                        for c in range(3):
                            dead = scr.tile([128, L], F16, tag="dead")
                            nc.vector.scalar_tensor_tensor(
                                out=dead[:], in0=vdxt[:, s, c], scalar=1.0,
                                in1=sg[:, s], op0=ALU.mult, op1=ALU.mult,
                                accum_out=facc[:, sh0 + s, c:c + 1])
                nc.gpsimd.dma_start(
                    A(o_fv, sl * 128 * NS * 3, [[NS * 3, 128], [1, NS * 3]]),
                    facc[:].rearrange("p s c -> p (s c)"))

            # ================= S family =================
            for sl, (L, base) in enumerate(((LSA, 0), (LSB, 128 * LSA))):
                sct = io.tile([128, 2, L], F16, tag=f"sct{sl}")
                nc.sync.dma_start(sct[:], A(t_scn, base, [[L, 128], [TS, 2], [1, L]]))
                sbc = lambda c: A(sct[:, c].tensor, sct[:, c].offset,
                                  [sct[:, c].ap[0], [0, NS], [1, L]])
                sacc = acc.tile([128, NS, 3], F32, tag=f"faccS{sl}")
                sdxt = io.tile([128, NS, 3, L], F16, tag=f"sdxt{sl}")
                nc.sync.dma_start(
                    sdxt[:], A(t_sdx, base,
                               [[L, 128], [3 * TS, NS], [TS, 3], [1, L]]))
                sxt = io.tile([128, NS, L], F16, tag=f"sxt{sl}")
                nc.sync.dma_start(
                    sxt[:], A(t_sx, base, [[L, 128], [TS, NS], [1, L]]))
                ts_ = scr.tile([128, NS, L], F16, tag="ts")
                nc.vector.tensor_mul(ts_[:], sxt[:], sbc(0))
                sg2 = scr.tile([128, NS, L], F16, tag="sg2")
                nc.vector.tensor_add(sg2[:], ts_[:], sbc(1))
                for s in range(NS):
                    for c in range(3):
                        dead = scr.tile([128, L], F16, tag="deadS")
                        nc.vector.scalar_tensor_tensor(
                            out=dead[:], in0=sdxt[:, s, c], scalar=1.0,
                            in1=sg2[:, s], op0=ALU.mult, op1=ALU.mult,
                            accum_out=sacc[:, s, c:c + 1])
                nc.gpsimd.dma_start(
                    A(o_fs, sl * 128 * NS * 3, [[NS * 3, 128], [1, NS * 3]]),
                    sacc[:].rearrange("p s c -> p (s c)"))

            # ================= small term-order energies =================
            # bond: e = K*(x-r0)^2   [125 x 32]
            nb_f = NS * NBc
            bkt = io.tile([125, nb_f // 125], F16, tag="bkt")
            nc.sync.dma_start(bkt[:], A(t_bK, 0, [[nb_f // 125, 125], [1, nb_f // 125]]))
            brt = io.tile([125, nb_f // 125], F16, tag="brt")
            nc.sync.dma_start(brt[:], A(t_bR, 0, [[nb_f // 125, 125], [1, nb_f // 125]]))
            lbt = io.tile([125, nb_f // 125], F16, tag="lbt")
            nc.sync.dma_start(lbt[:], A(t_lb, 0, [[nb_f // 125, 125], [1, nb_f // 125]]))
            dd = scr.tile([125, nb_f // 125], F16, tag="sm0")
            nc.vector.tensor_sub(dd[:], lbt[:], brt[:])
            kd = scr.tile([125, nb_f // 125], F16, tag="sm1")
            nc.gpsimd.tensor_mul(kd[:], dd[:], bkt[:])
            eb = scr.tile([125, nb_f // 125], F16, tag="sm2")
            nc.vector.tensor_mul(eb[:], kd[:], dd[:])
            nc.gpsimd.dma_start(A(o_eb, 0, [[nb_f // 125, 125], [1, nb_f // 125]]), eb[:])
            # angle [125 x 64]
            na_f = NS * NAc
            akt = io.tile([125, na_f // 125], F16, tag="akt")
            nc.sync.dma_start(akt[:], A(t_aK, 0, [[na_f // 125, 125], [1, na_f // 125]]))
            art = io.tile([125, na_f // 125], F16, tag="art")
            nc.sync.dma_start(art[:], A(t_aT, 0, [[na_f // 125, 125], [1, na_f // 125]]))
            tht = io.tile([125, na_f // 125], F16, tag="tht")
            nc.sync.dma_start(tht[:], A(t_th, 0, [[na_f // 125, 125], [1, na_f // 125]]))
            da = scr.tile([125, na_f // 125], F16, tag="sm3")
            nc.vector.tensor_sub(da[:], tht[:], art[:])
            ka = scr.tile([125, na_f // 125], F16, tag="sm4")
            nc.gpsimd.tensor_mul(ka[:], da[:], akt[:])
            ea = scr.tile([125, na_f // 125], F16, tag="sm5")
            nc.vector.tensor_mul(ea[:], ka[:], da[:])
            nc.gpsimd.dma_start(A(o_ea, 0, [[na_f // 125, 125], [1, na_f // 125]]), ea[:])
            # torsion energy: e = sum_n cos_n * p_n   [125 x 96 x 8]
            nt8 = NS * NTc * 8
            cw = nt8 // 125 // 8        # 96
            sctt = io.tile([125, cw, 8], F16, tag="sctt")
            nc.sync.dma_start(sctt[:], A(t_sc, 0, [[cw * 8, 125], [8, cw], [1, 8]]))
            ptt = io.tile([125, cw, 4], F16, tag="ptt")
            nc.sync.dma_start(ptt[:], A(t_pt, 0, [[cw * 4, 125], [4, cw], [1, 4]]))
            cos_ap = A(sctt[:].tensor, sctt[:].offset + 1,
                       [sctt[:].ap[0], [8, cw], [2, 4]])
            prod = scr.tile([125, cw, 4], F16, tag="sm6")
            nc.vector.tensor_mul(prod[:], cos_ap, ptt[:])
            et = scr.tile([125, cw], F32, tag="sm7")
            nc.vector.reduce_sum(et[:], prod[:], axis=AX.X)
            nc.gpsimd.dma_start(A(o_et, 0, [[cw, 125], [1, cw]]), et[:])
            # imptors energy: out = (c2-1)*ki = -E  (host negates)
            ni_f = NS * NIc
            kit = io.tile([125, ni_f // 125], F16, tag="kit")
            nc.sync.dma_start(kit[:], A(t_kT, 0, [[ni_f // 125, 125], [1, ni_f // 125]]))
            c2t = io.tile([125, ni_f // 125], F16, tag="c2t")
            nc.sync.dma_start(c2t[:], A(t_c2, 0, [[ni_f // 125, 125], [1, ni_f // 125]]))
            ei = scr.tile([125, ni_f // 125], F16, tag="sm8")
            nc.vector.scalar_tensor_tensor(
                out=ei[:], in0=c2t[:], scalar=1.0, in1=kit[:],
                op0=ALU.subtract, op1=ALU.mult)
            nc.gpsimd.dma_start(A(o_ei, 0, [[ni_f // 125, 125], [1, ni_f // 125]]), ei[:])

    nc.finalize()
    _NC_CACHE[key] = nc
    return nc


# ----------------------------------------------------------------------------
# Entry points
# ----------------------------------------------------------------------------

def _in_maps(host):
    maps = []
    for c in range(NCORES):
        sm = host["smalls"][c]
        maps.append({
            "vdx": host["vdx"][c], "vr": host["vr"][c], "vcon": host["vcon"][c],
            "sdx": host["sdx"][c], "sx": host["sx"][c], "scon": host["scon"][c],
            "lb": sm["lb"], "th": sm["th"], "sc": sm["sc"].reshape(NS, -1),
            "c2": sm["c2"], "bK": sm["bK"], "bR": sm["bR"],
            "aK": sm["aK"], "aT": sm["aT"], "pt": sm["pt"], "kT": sm["kT"],
        })
    return maps


def _assemble(results, meta):
    orderV, orderS = meta["orderV"], meta["orderS"]
    f32 = np.float32

    # small energies (term-sharded)
    e_bond = np.concatenate([r["o_eb"] for r in results], axis=1).astype(f32)
    e_angle = np.concatenate([r["o_ea"] for r in results], axis=1).astype(f32)
    e_tors = np.concatenate([r["o_et"] for r in results], axis=1).astype(f32)
    e_impt = (-np.concatenate([r["o_ei"] for r in results], axis=1)).astype(f32)

    # vdw/coulomb energies: gather end-0 entries
    ev_all = np.stack([r["o_ev"] for r in results])   # (8, NS, TV) f16
    ec_all = np.stack([r["o_ec"] for r in results])
    ci, pi = meta["ev_core"], meta["ev_pos"]
    e_vdw = (ev_all[ci, :, pi].T.astype(f32)) / 12.0  # (NS, NV)
    e_charge = ec_all[ci, :, pi].T.astype(f32)

    # forces
    fv = np.stack([r["o_fv"] for r in results])       # (8, 2, 128, NS, 3)
    fs = np.stack([r["o_fs"] for r in results])
    rk = np.arange(2048)
    t = rk >> 7
    core = np.where(t < 8, t, 15 - t)
    sidx = (t >= 8).astype(np.int64)
    row = rk & 127
    force = np.zeros((NS, N_ATOMS, 3), f32)
    fv_m = fv[core, sidx, row]                        # (2048, NS, 3)
    fs_m = fs[core, sidx, row]
    force[:, orderV] = -fv_m[:N_ATOMS].transpose(1, 0, 2)
    tmp = np.zeros((NS, N_ATOMS, 3), f32)
    tmp[:, orderS] = fs_m[:N_ATOMS].transpose(1, 0, 2)
    force += tmp

    return np.concatenate([
        e_bond, e_angle, np.zeros((NS, 1), f32), e_vdw, e_charge,
        e_tors, e_impt, force.reshape(NS, -1),
    ], axis=1)


def run(inputs, trace=False):
    host, meta = _host_prep(inputs)
    nc = _build_nc(meta["LA"], meta["LB"], meta["TV"],
                   meta["LSA"], meta["LSB"], meta["TS"])
    res = run_bass_kernel_spmd(nc, _in_maps(host), list(range(NCORES)),
                               trace=trace)
    return _assemble(res.results, meta), res


def kernel(**inputs) -> np.ndarray:
    out, _ = run(inputs)
    return out


# revision 9
# speedup vs baseline: 2.0313x; 1.4819x over previous
"""Trainium2 Bass kernel for nn_ComputeEnergyForce (force-field energy+force).

Strategy (v3): transposed scatter layout + PE reduction
-------------------------------------------------------
Entry-sharding over 8 cores (250 atoms each, all 16 shots).  Scatter
entries are laid out TRANSPOSED: slot index on the partition axis (K=128
rows), (atom, subcolumn) on the free axis.  The per-atom force reduction
is then a column sum = PE matmul with a ones moving vector:
    out[m, 1] = sum_k P[k, m],  P = dx_c * sg   (lhsT = P chunk, rhs = ones)
which runs on the otherwise-idle Tensor engine and accumulates in PSUM.
The DVE only materializes the product P at fp16 2x; the old
scalar_tensor_tensor accumulation (1x, 58us+) disappears.

Per-entry force scalar (V = vdw+coulomb pairs, s = force scalar):
    lr = Ln(r); w = Exp(-lr); w6 = Exp(-6lr)          (Scalar engine)
    u = w6*S; t = Sq(u-.5); t3 = Sq(u-1)              (V / Scalar)
    q1 = (t-1/4)*A; t2 = C*w; d = t2+q1; sg = w*d     (= -s; Vector)
Ln's are phase-batched per chunk so the ACT table set loads ~4x total
instead of every Ln<->Exp transition.  GpSimd does no elementwise work
(shares an SBUF port with the DVE; concurrent use slows 2x DVE ops 3x).

Energies ride the force pass: E_charge = t2 (output per-entry, host
gathers end-0 entry); E_vdw = A*(t3-1)/12 computed on HOST from the t3
output (pure per-pair affine on gathered values).  S family
(bond/angle/imptors/torsion-harmonics): sg = a*x+b, same PE reduction.
Small term-order passes emit bond/angle/torsion/imptors energies.
"""

import numpy as np

import concourse.bass as bass
import concourse.bacc as bacc
import concourse.mybir as mybir
from concourse import tile
from concourse.bass_utils import run_bass_kernel_spmd

F32 = mybir.dt.float32
F16 = mybir.dt.float16
AF = mybir.ActivationFunctionType
ALU = mybir.AluOpType
AX = mybir.AxisListType

NS, N_ATOMS = 16, 2000
NB, NA, NV, NT, NI = 2000, 4000, 400000, 6000, 1000
CHARGE = 18.222615
NCORES = 8
APC = N_ATOMS // NCORES    # atoms per core
G = 4                      # shots per group (V family)
NSG = NS // G
GS = 8                     # shots per group (S family)
NSGS = NS // GS
NCH = 2                    # column chunks (V family)
NBc, NAc, NTc, NIc = NB // 8, NA // 8, NT // 8, NI // 8
K = 128                    # slot rows (partition/contraction dim)


# ----------------------------------------------------------------------------
# Host-side packing: transposed column layout
# ----------------------------------------------------------------------------

def _pack_cols(atom_ids, round_to):
    """Assign each entry to (core, row, col).  Atom a -> core a//APC; its
    entries fill ceil(cnt/K) dedicated columns of K rows.  Returns per-entry
    (core, row, col), common per-core column count C, and colmap (8, C)
    giving the atom id per column (-1 = pad)."""
    E = len(atom_ids)
    cnt = np.bincount(atom_ids, minlength=N_ATOMS)
    ncols_atom = np.maximum(1, -(-cnt // K))
    core_of_atom = np.arange(N_ATOMS) // APC
    colbase = np.zeros(N_ATOMS, np.int64)
    Cs = np.zeros(NCORES, np.int64)
    for c in range(NCORES):
        sel = slice(c * APC, (c + 1) * APC)
        nc_ = ncols_atom[sel]
        colbase[sel] = np.concatenate([[0], np.cumsum(nc_)[:-1]])
        Cs[c] = nc_.sum()
    C = -(-int(Cs.max()) // round_to) * round_to
    # per-entry slot j within its atom
    order = np.argsort(atom_ids, kind="stable")
    j_sorted = np.arange(E) - np.concatenate([[0], np.cumsum(cnt)])[:-1][atom_ids[order]]
    j = np.empty(E, np.int64)
    j[order] = j_sorted
    row = j % K
    col = colbase[atom_ids] + j // K
    core = core_of_atom[atom_ids]
    colmap = np.full((NCORES, C), -1, np.int64)
    for c in range(NCORES):
        sel = slice(c * APC, (c + 1) * APC)
        aidx = np.repeat(np.arange(c * APC, (c + 1) * APC), ncols_atom[sel])
        colmap[c, :len(aidx)] = aidx
    return core, row, col, C, colmap


def _host_prep(inp):
    f = lambda k: np.asarray(inp[k], dtype=np.float32)
    ii = lambda k: np.asarray(inp[k], dtype=np.int64)
    h16 = np.float16

    length_bond = f("length_bond"); theta_angle = f("theta_angle")
    length_vdw = f("length_vdw"); sin_cos = f("sin_cos_torsion")
    cos2 = f("cos2_imptors")
    vdw14 = f("vdw14"); charge14 = f("charge14")
    pb = f("paras_bond"); pa = f("paras_angle"); pv = f("paras_vdw")
    pc = f("paras_charge"); ptor = f("paras_torsion"); pimp = f("paras_imptors")
    dlb = f("dlength_bond"); dta = f("dtheta_angle"); dlv = f("dlength_vdw")
    dtt = f("dtheta_torsion"); dci = f("dcos2_imptors")
    nb = ii("nonbonded"); b_idx = ii("bond_index"); a_idx = ii("angle_index")
    nb_idx = ii("nonbonded_index"); t_idx = ii("torsion_index")
    i_idx = ii("imptors_index")

    i, j = nb[0], nb[1]
    sig6 = (pv[i, 0].astype(np.float64) + pv[j, 0].astype(np.float64)) ** 6
    A12 = 12.0 * (pv[i, 1].astype(np.float64) / 10.0) * (pv[j, 1] / 10.0) * vdw14
    Ccc = (CHARGE / 10.0) ** 2 * pc[i].astype(np.float64) * pc[j] * charge14

    # ---------------- V family ----------------
    EV = 2 * NV
    av = nb_idx.reshape(-1)
    tv = np.arange(EV) >> 1
    ceV, rwV, clV, CV, cmapV = _pack_cols(av, round_to=8 * NCH)
    CC = CV // NCH             # chain chunk width
    MV = CC // 4               # matmul subchunk width

    vdx = np.zeros((NCORES, NS, 3, K, CV), h16)
    vdx[ceV, :, :, rwV, clV] = dlv.reshape(NS, EV, 3).transpose(1, 0, 2)
    vr = np.ones((NCORES, NS, K, CV), h16)
    vr[ceV, :, rwV, clV] = length_vdw[:, tv].T
    vcon = np.zeros((NCORES, 3, K, CV), h16)
    vcon[ceV, 0, rwV, clV] = sig6[tv]
    vcon[ceV, 1, rwV, clV] = A12[tv]
    vcon[ceV, 2, rwV, clV] = Ccc[tv]
    e0 = np.arange(0, EV, 2)
    evg = (ceV[e0], rwV[e0], clV[e0])     # energy gather index (end-0 entry)

    # ---------------- S family ----------------
    K_ = pb[:, 0].astype(np.float64) * 100.0
    r0 = pb[:, 1].astype(np.float64)
    Ka = pa[:, 0].astype(np.float64) * 10.0
    th0 = pa[:, 1].astype(np.float64) * (np.pi / 10.0)
    ki = pimp[:, 0].astype(np.float64)
    coeff = ptor.astype(np.float64) * np.arange(1, 5, dtype=np.float64)[None]

    e_b = np.arange(2 * NB) >> 1
    e_a = np.arange(3 * NA) // 3
    e_i = np.arange(4 * NI) >> 2
    ntt = 4 * NT
    tt = np.arange(ntt) >> 2
    tt_rep = np.repeat(tt, 4)
    n_rep = np.tile(np.arange(4), ntt)

    aS = np.concatenate([
        b_idx.reshape(-1), a_idx.reshape(-1), i_idx.reshape(-1),
        np.repeat(t_idx.reshape(-1), 4),
    ])
    caS = np.concatenate([
        (2.0 * K_)[e_b], (2.0 * Ka)[e_a], np.zeros(4 * NI),
        -coeff[tt_rep, n_rep],
    ])
    cbS = np.concatenate([
        (-2.0 * K_ * r0)[e_b], (-2.0 * Ka * th0)[e_a], -ki[e_i],
        np.zeros(4 * ntt),
    ])
    off_th, off_sc, off_z = NB, NB + NA, NB + NA + NT * 8
    xiS = np.concatenate([
        e_b, off_th + e_a, np.full(4 * NI, off_z, np.int64),
        off_sc + tt_rep * 8 + 2 * n_rep,
    ])
    XS = np.concatenate([
        length_bond, theta_angle, sin_cos.reshape(NS, -1),
        np.zeros((NS, 1), np.float32),
    ], axis=1)
    dxS = np.concatenate([
        dlb.reshape(NS, 2 * NB, 3), dta.reshape(NS, 3 * NA, 3),
        dci.reshape(NS, 4 * NI, 3),
        np.repeat(dtt.reshape(NS, ntt, 3), 4, axis=1),
    ], axis=1)

    ceS, rwS, clS, CS, cmapS = _pack_cols(aS, round_to=4)
    MS = CS // 4

    sdx = np.zeros((NCORES, NS, 3, K, CS), h16)
    sdx[ceS, :, :, rwS, clS] = dxS.transpose(1, 0, 2)
    sx = np.zeros((NCORES, NS, K, CS), h16)
    sx[ceS, :, rwS, clS] = XS[:, xiS].T
    scon = np.zeros((NCORES, 2, K, CS), h16)
    scon[ceS, 0, rwS, clS] = caS
    scon[ceS, 1, rwS, clS] = cbS

    # ---------------- small term-order energy slices ----------------
    smalls = []
    for c in range(NCORES):
        sb, sa = slice(c * NBc, (c + 1) * NBc), slice(c * NAc, (c + 1) * NAc)
        st, si = slice(c * NTc, (c + 1) * NTc), slice(c * NIc, (c + 1) * NIc)
        smalls.append(dict(
            lb=length_bond[:, sb].astype(h16),
            th=theta_angle[:, sa].astype(h16),
            sc=np.ascontiguousarray(sin_cos[:, st]).astype(h16),
            c2=cos2[:, si].astype(h16),
            bK=np.tile(K_[sb], NS).astype(h16),
            bR=np.tile(r0[sb], NS).astype(h16),
            aK=np.tile(Ka[sa], NS).astype(h16),
            aT=np.tile(th0[sa], NS).astype(h16),
            pt=np.tile(ptor[st].reshape(-1), NS).astype(h16),
            kT=np.tile(ki[si], NS).astype(h16),
        ))

    host = dict(vdx=vdx, vr=vr, vcon=vcon, sdx=sdx, sx=sx, scon=scon,
                smalls=smalls)
    meta = dict(CV=CV, CC=CC, MV=MV, CS=CS, MS=MS,
                cmapV=cmapV, cmapS=cmapS, evg=evg,
                A12=A12.astype(np.float32), Ccc=Ccc.astype(np.float32))
    return host, meta


# ----------------------------------------------------------------------------
# Device kernel
# ----------------------------------------------------------------------------

_NC_CACHE = {}


def _build_nc(CV, CC, MV, CS, MS):
    key = (CV, CS)
    if key in _NC_CACHE:
        return _NC_CACHE[key]

    nc = bacc.Bacc("TRN2")
    dp = lambda n, s, dt=F16, o=False: nc.declare_dram_parameter(
        n, list(s), dt, isOutput=o)

    t_vdx = dp("vdx", (NS, 3, K, CV)); t_vr = dp("vr", (NS, K, CV))
    t_vc = dp("vcon", (3, K, CV))
    t_sdx = dp("sdx", (NS, 3, K, CS)); t_sx = dp("sx", (NS, K, CS))
    t_scn = dp("scon", (2, K, CS))
    t_lb = dp("lb", (NS, NBc)); t_th = dp("th", (NS, NAc))
    t_sc = dp("sc", (NS, NTc * 8)); t_c2 = dp("c2", (NS, NIc))
    t_bK = dp("bK", (NS * NBc,)); t_bR = dp("bR", (NS * NBc,))
    t_aK = dp("aK", (NS * NAc,)); t_aT = dp("aT", (NS * NAc,))
    t_pt = dp("pt", (NS * NTc * 4,)); t_kT = dp("kT", (NS * NIc,))

    o_t3 = dp("o_t3", (NS, K, CV), o=True)
    o_ec = dp("o_ec", (NS, K, CV), o=True)
    o_fv = dp("o_fv", (NCH, MV, NS, 3, 4), F32, True)
    o_fs = dp("o_fs", (NSGS, MS, GS, 3, 4), F32, True)
    o_eb = dp("o_eb", (NS, NBc), o=True); o_ea = dp("o_ea", (NS, NAc), o=True)
    o_et = dp("o_et", (NS, NTc), o=True); o_ei = dp("o_ei", (NS, NIc), o=True)

    A = bass.AP

    with tile.TileContext(nc) as tc:
        with tc.tile_pool(name="io", bufs=2) as io, \
             tc.tile_pool(name="io1", bufs=1) as io1, \
             tc.tile_pool(name="lrp", bufs=2) as lrp, \
             tc.tile_pool(name="scr", bufs=2) as scr, \
             tc.tile_pool(name="scr1", bufs=1) as scr1, \
             tc.tile_pool(name="ps", bufs=2, space=bass.MemorySpace.PSUM) as psp:

            ones = io.tile([K, 1], F16, tag="ones")
            nc.vector.memset(ones[:], 1.0)
            cm05 = io.tile([K, 1], F32, tag="cm05")
            nc.vector.memset(cm05[:], -0.5)
            cm10 = io.tile([K, 1], F32, tag="cm10")
            nc.vector.memset(cm10[:], -1.0)

            # ================= V family =================
            for ch in range(NCH):
                cb = ch * CC
                vct = io.tile([K, 3, CC], F16, tag="vct")
                nc.sync.dma_start(
                    vct[:], A(t_vc, cb, [[CV, K], [K * CV, 3], [1, CC]]))
                bc = lambda c: A(vct[:, c].tensor, vct[:, c].offset,
                                 [vct[:, c].ap[0], [0, G], [1, CC]])
                psv = psp.tile([MV, NS, 3, 4], F32, tag="psv")
                # --- Ln phase (batched so ACT table set isn't thrashed) ---
                lrs = []
                for g in range(NSG):
                    vrt = io.tile([K, G, CC], F16, tag="vrt")
                    nc.sync.dma_start(
                        vrt[:], A(t_vr, g * G * K * CV + cb,
                                  [[CV, K], [K * CV, G], [1, CC]]))
                    lr = lrp.tile([K, G, CC], F16, tag=f"lr{g}")
                    nc.scalar.activation(lr[:], vrt[:], AF.Ln)
                    lrs.append(lr)
                # --- chain + PE per shot group ---
                for g in range(NSG):
                    sh0 = g * G
                    vdxt = io.tile([K, G, 3, CC], F16, tag="vdxt")
                    nc.sync.dma_start(
                        vdxt[:], A(t_vdx, sh0 * 3 * K * CV + cb,
                                   [[CV, K], [3 * K * CV, G], [K * CV, 3], [1, CC]]))
                    w = scr.tile([K, G, CC], F16, tag="w")
                    nc.scalar.activation(w[:], lrs[g][:], AF.Exp, scale=-1.0)
                    w6 = scr.tile([K, G, CC], F16, tag="w6")
                    nc.scalar.activation(w6[:], lrs[g][:], AF.Exp, scale=-6.0)
                    u = scr1.tile([K, G, CC], F16, tag="u")
                    nc.vector.tensor_mul(u[:], w6[:], bc(0))
                    t = scr1.tile([K, G, CC], F16, tag="t")
                    nc.scalar.activation(t[:], u[:], AF.Square, bias=cm05[:])
                    t3 = scr1.tile([K, G, CC], F16, tag="t3")
                    nc.scalar.activation(t3[:], u[:], AF.Square, bias=cm10[:])
                    nc.gpsimd.dma_start(
                        A(o_t3, sh0 * K * CV + cb, [[CV, K], [K * CV, G], [1, CC]]),
                        t3[:])
                    q1 = scr1.tile([K, G, CC], F16, tag="q1")
                    nc.vector.scalar_tensor_tensor(
                        out=q1[:], in0=t[:], scalar=0.25, in1=bc(1),
                        op0=ALU.subtract, op1=ALU.mult)
                    t2 = scr1.tile([K, G, CC], F16, tag="t2")
                    nc.vector.tensor_mul(t2[:], w[:], bc(2))
                    nc.gpsimd.dma_start(
                        A(o_ec, sh0 * K * CV + cb, [[CV, K], [K * CV, G], [1, CC]]),
                        t2[:])
                    d = scr1.tile([K, G, CC], F16, tag="d")
                    nc.vector.tensor_add(d[:], t2[:], q1[:])
                    sg = scr.tile([K, G, CC], F16, tag="sg")
                    nc.vector.tensor_mul(sg[:], w[:], d[:])
                    P = scr.tile([K, G, 3, CC], F16, tag="P")
                    sga = sg[:]
                    nc.vector.tensor_mul(
                        P[:], vdxt[:],
                        A(sga.tensor, sga.offset, [sga.ap[0], [CC, G], [0, 3], [1, CC]]))
                    for s in range(G):
                        for c in range(3):
                            for sub in range(4):
                                nc.tensor.matmul(
                                    psv[:, sh0 + s, c, sub:sub + 1],
                                    P[:, s, c, sub * MV:(sub + 1) * MV],
                                    ones[:])
                fres = scr.tile([MV, NS * 3 * 4], F32, tag="fres")
                nc.vector.tensor_copy(
                    fres[:], psv[:].rearrange("p s c k -> p (s c k)"))
                nc.gpsimd.dma_start(
                    A(o_fv, ch * MV * NS * 12, [[NS * 12, MV], [1, NS * 12]]),
                    fres[:])

            # ================= S family =================
            sct = io.tile([K, 2, CS], F16, tag="sct")
            nc.sync.dma_start(sct[:], A(t_scn, 0, [[CS, K], [K * CS, 2], [1, CS]]))
            sbc = lambda c: A(sct[:, c].tensor, sct[:, c].offset,
                              [sct[:, c].ap[0], [0, GS], [1, CS]])
            for sgi in range(NSGS):
                sh0 = sgi * GS
                sdxt = io1.tile([K, GS, 3, CS], F16, tag="sdxt")
                nc.sync.dma_start(
                    sdxt[:], A(t_sdx, sh0 * 3 * K * CS,
                               [[CS, K], [3 * K * CS, GS], [K * CS, 3], [1, CS]]))
                sxt = io.tile([K, GS, CS], F16, tag="sxt")
                nc.sync.dma_start(
                    sxt[:], A(t_sx, sh0 * K * CS, [[CS, K], [K * CS, GS], [1, CS]]))
                ts_ = scr1.tile([K, GS, CS], F16, tag="ts")
                nc.vector.tensor_mul(ts_[:], sxt[:], sbc(0))
                sg2 = scr1.tile([K, GS, CS], F16, tag="sg2")
                nc.vector.tensor_add(sg2[:], ts_[:], sbc(1))
                PS = scr1.tile([K, GS, 3, CS], F16, tag="PS")
                s2a = sg2[:]
                nc.vector.tensor_mul(
                    PS[:], sdxt[:],
                    A(s2a.tensor, s2a.offset, [s2a.ap[0], [CS, GS], [0, 3], [1, CS]]))
                pss = psp.tile([MS, GS, 3, 4], F32, tag="pss")
                for s in range(GS):
                    for c in range(3):
                        for sub in range(4):
                            nc.tensor.matmul(
                                pss[:, s, c, sub:sub + 1],
                                PS[:, s, c, sub * MS:(sub + 1) * MS],
                                ones[:])
                fs_ = scr.tile([MS, GS * 3 * 4], F32, tag="fs")
                nc.vector.tensor_copy(
                    fs_[:], pss[:].rearrange("p s c k -> p (s c k)"))
                nc.gpsimd.dma_start(
                    A(o_fs, sgi * MS * GS * 12, [[GS * 12, MS], [1, GS * 12]]),
                    fs_[:])

            # ================= small term-order energies =================
            nb_f = NS * NBc
            bkt = io.tile([125, nb_f // 125], F16, tag="bkt")
            nc.sync.dma_start(bkt[:], A(t_bK, 0, [[nb_f // 125, 125], [1, nb_f // 125]]))
            brt = io.tile([125, nb_f // 125], F16, tag="brt")
            nc.sync.dma_start(brt[:], A(t_bR, 0, [[nb_f // 125, 125], [1, nb_f // 125]]))
            lbt = io.tile([125, nb_f // 125], F16, tag="lbt")
            nc.sync.dma_start(lbt[:], A(t_lb, 0, [[nb_f // 125, 125], [1, nb_f // 125]]))
            dd = scr.tile([125, nb_f // 125], F16, tag="sm0")
            nc.vector.tensor_sub(dd[:], lbt[:], brt[:])
            kd = scr.tile([125, nb_f // 125], F16, tag="sm1")
            nc.vector.tensor_mul(kd[:], dd[:], bkt[:])
            eb = scr.tile([125, nb_f // 125], F16, tag="sm2")
            nc.vector.tensor_mul(eb[:], kd[:], dd[:])
            nc.gpsimd.dma_start(A(o_eb, 0, [[nb_f // 125, 125], [1, nb_f // 125]]), eb[:])
            na_f = NS * NAc
            akt = io.tile([125, na_f // 125], F16, tag="akt")
            nc.sync.dma_start(akt[:], A(t_aK, 0, [[na_f // 125, 125], [1, na_f // 125]]))
            art = io.tile([125, na_f // 125], F16, tag="art")
            nc.sync.dma_start(art[:], A(t_aT, 0, [[na_f // 125, 125], [1, na_f // 125]]))
            tht = io.tile([125, na_f // 125], F16, tag="tht")
            nc.sync.dma_start(tht[:], A(t_th, 0, [[na_f // 125, 125], [1, na_f // 125]]))
            da = scr.tile([125, na_f // 125], F16, tag="sm3")
            nc.vector.tensor_sub(da[:], tht[:], art[:])
            ka = scr.tile([125, na_f // 125], F16, tag="sm4")
            nc.vector.tensor_mul(ka[:], da[:], akt[:])
            ea = scr.tile([125, na_f // 125], F16, tag="sm5")
            nc.vector.tensor_mul(ea[:], ka[:], da[:])
            nc.gpsimd.dma_start(A(o_ea, 0, [[na_f // 125, 125], [1, na_f // 125]]), ea[:])
            nt8 = NS * NTc * 8
            cw = nt8 // 125 // 8
            sctt = io.tile([125, cw, 8], F16, tag="sctt")
            nc.sync.dma_start(sctt[:], A(t_sc, 0, [[cw * 8, 125], [8, cw], [1, 8]]))
            ptt = io.tile([125, cw, 4], F16, tag="ptt")
            nc.sync.dma_start(ptt[:], A(t_pt, 0, [[cw * 4, 125], [4, cw], [1, 4]]))
            cos_ap = A(sctt[:].tensor, sctt[:].offset + 1,
                       [sctt[:].ap[0], [8, cw], [2, 4]])
            prod = scr.tile([125, cw, 4], F16, tag="sm6")
            nc.vector.tensor_mul(prod[:], cos_ap, ptt[:])
            et = scr.tile([125, cw], F32, tag="sm7")
            nc.vector.reduce_sum(et[:], prod[:], axis=AX.X)
            nc.gpsimd.dma_start(A(o_et, 0, [[cw, 125], [1, cw]]), et[:])
            ni_f = NS * NIc
            kit = io.tile([125, ni_f // 125], F16, tag="kit")
            nc.sync.dma_start(kit[:], A(t_kT, 0, [[ni_f // 125, 125], [1, ni_f // 125]]))
            c2t = io.tile([125, ni_f // 125], F16, tag="c2t")
            nc.sync.dma_start(c2t[:], A(t_c2, 0, [[ni_f // 125, 125], [1, ni_f // 125]]))
            ei = scr.tile([125, ni_f // 125], F16, tag="sm8")
            nc.vector.scalar_tensor_tensor(
                out=ei[:], in0=c2t[:], scalar=1.0, in1=kit[:],
                op0=ALU.subtract, op1=ALU.mult)
            nc.gpsimd.dma_start(A(o_ei, 0, [[ni_f // 125, 125], [1, ni_f // 125]]), ei[:])

    nc.finalize()
    _NC_CACHE[key] = nc
    return nc


# ----------------------------------------------------------------------------
# Entry points
# ----------------------------------------------------------------------------

def _in_maps(host):
    maps = []
    for c in range(NCORES):
        sm = host["smalls"][c]
        maps.append({
            "vdx": host["vdx"][c], "vr": host["vr"][c], "vcon": host["vcon"][c],
            "sdx": host["sdx"][c], "sx": host["sx"][c], "scon": host["scon"][c],
            "lb": sm["lb"], "th": sm["th"], "sc": sm["sc"].reshape(NS, -1),
            "c2": sm["c2"], "bK": sm["bK"], "bR": sm["bR"],
            "aK": sm["aK"], "aT": sm["aT"], "pt": sm["pt"], "kT": sm["kT"],
        })
    return maps


def _assemble(results, meta):
    f32 = np.float32
    CV, CC, MV, CS, MS = (meta[k] for k in ("CV", "CC", "MV", "CS", "MS"))

    e_bond = np.concatenate([r["o_eb"] for r in results], axis=1).astype(f32)
    e_angle = np.concatenate([r["o_ea"] for r in results], axis=1).astype(f32)
    e_tors = np.concatenate([r["o_et"] for r in results], axis=1).astype(f32)
    e_impt = (-np.concatenate([r["o_ei"] for r in results], axis=1)).astype(f32)

    # vdw/coulomb energies from per-entry outputs (end-0 entry of each pair)
    ec_all = np.stack([r["o_ec"] for r in results])    # (8, NS, K, CV)
    t3_all = np.stack([r["o_t3"] for r in results])
    ci, ri, li = meta["evg"]
    e_charge = ec_all[ci, :, ri, li].T.astype(f32)
    t3g = t3_all[ci, :, ri, li].T.astype(f32)
    e_vdw = meta["A12"][None] * (t3g - 1.0) / 12.0

    # forces: column sums -> atoms.  col id = ch*CC + sub*MV + p  (V family)
    force = np.zeros((N_ATOMS, NS, 3), f32)
    fv = np.stack([r["o_fv"] for r in results])        # (8, NCH, MV, NS, 3, 4)
    cols = (np.arange(NCH)[:, None, None] * CC
            + np.arange(4)[None, :, None] * MV
            + np.arange(MV)[None, None, :])            # (NCH, 4, MV)
    for c in range(NCORES):
        amap = meta["cmapV"][c][cols.reshape(-1)]      # (NCH*4*MV,)
        vals = fv[c].transpose(0, 4, 1, 2, 3).reshape(-1, NS, 3)
        ok = amap >= 0
        np.add.at(force, amap[ok], -vals[ok])
    fs = np.stack([r["o_fs"] for r in results])        # (8, NSGS, MS, GS, 3, 4)
    colsS = (np.arange(4)[:, None] * MS + np.arange(MS)[None, :])
    for c in range(NCORES):
        amap = meta["cmapS"][c][colsS.reshape(-1)]     # (4*MS,)
        vals = fs[c].transpose(4, 1, 0, 2, 3).reshape(4 * MS, NS, 3)
        ok = amap >= 0
        np.add.at(force, amap[ok], vals[ok])
    force = force.transpose(1, 0, 2)

    return np.concatenate([
        e_bond, e_angle, np.zeros((NS, 1), f32), e_vdw, e_charge,
        e_tors, e_impt, force.reshape(NS, -1),
    ], axis=1)


def run(inputs, trace=False):
    host, meta = _host_prep(inputs)
    nc = _build_nc(meta["CV"], meta["CC"], meta["MV"], meta["CS"], meta["MS"])
    res = run_bass_kernel_spmd(nc, _in_maps(host), list(range(NCORES)),
                               trace=trace)
    return _assemble(res.results, meta), res


def kernel(**inputs) -> np.ndarray:
    out, _ = run(inputs)
    return out
